# revision 1
# baseline (speedup 1.0000x reference)
"""Trainium2 Bass kernel for nn_ConditioningEncoder (6-layer attention encoder).

Strategy: data-parallel over batch (B=8 -> 1 batch element per NeuronCore).
All matmuls run in float32r (tf32-class precision at bf16-class speed).

Per-core computation (C=1024 channels, L=1024 positions, 16 heads, dh=64):
  x = init_w @ speech + init_b                        [C, L]
  6x attention blocks:
    h = GroupNorm32(x) * gn_w + gn_b                  (f32r)
    q, k, v = per-channel projections of h (head-major channel order)
    vT = PE-transpose of v per head + ones column (softmax denominator trick)
    per head: S^T[s,t] = k_h^T q_h; E = exp(S^T/8);
              AV psum[65,t] = [v;1]^T E accumulated over s-chunks
              attn[c,t] = AV[0:64] * (1/AV[64]) broadcast via K=1 matmul
    x += proj_w @ attn + proj_b
  out = x[:, 0]

Layer 5 computes q/attention/proj only for t=0 (only column 0 is returned).
"""
import sys

sys.path.insert(0, "/opt/trn_rl_repo")

from contextlib import ExitStack

import numpy as np

import concourse.bass as bass
import concourse.tile as tile
from concourse import bacc, mybir
from concourse.bass_utils import run_bass_kernel_spmd

f32 = mybir.dt.float32
f32r = mybir.dt.float32r
AF = mybir.ActivationFunctionType
Alu = mybir.AluOpType

B, SPEC, L = 8, 80, 1024
C, H, DH, NL, NG = 1024, 16, 64, 6, 32
CC = C // 128          # channel chunks per full width
EPS = 1e-5
NCORES = 8

LAST_RESULT = None     # test harness reads exec_time from here
_CACHE = {}


def _build():
    nc = bacc.Bacc("TRN2", target_bir_lowering=False, debug=False,
                   num_devices=NCORES)

    dr = {}
    def din(name, shape, dt):
        dr[name] = nc.dram_tensor(name, shape, dt, kind="ExternalInput").ap()

    din("speech", [SPEC, L], f32r)
    din("initw", [SPEC, CC, 128], f32r)
    din("initb", [128, CC], f32)
    for w in ("wq", "wk", "wv", "wp"):
        din(w, [NL, CC, 128, CC, 128], f32r)
    for b in ("bq", "bk", "bv", "bp", "gnw", "gnb"):
        din(b, [128, NL, CC], f32)
    din("ind", [128, 4], f32r)
    din("indt", [4, 128], f32r)
    din("ident", [128, 128], f32r)
    din("onesr", [1, 128], f32r)
    din("onesc", [128, 1], f32r)
    din("epsc", [128, 1], f32)
    out_d = nc.dram_tensor("out", [C], f32, kind="ExternalOutput").ap()
    taps = {}
    import os as _os
    TAPS = bool(int(_os.environ.get("KTAPS", "0")))
    if TAPS:
        for tn, shp in [("t_xi", [128, CC, L]), ("t_h0", [128, CC, L]),
                        ("t_q0", [128, CC // 2, L]), ("t_k0", [128, CC // 2, L]),
                        ("t_at0", [128, CC, L]), ("t_x0", [128, CC, L])]:
            taps[tn] = nc.dram_tensor(tn, shp, f32, kind="ExternalOutput").ap()

    with tile.TileContext(nc) as tc, ExitStack() as ctx:
        cst = ctx.enter_context(tc.tile_pool(name="cst", bufs=1))
        big = ctx.enter_context(tc.tile_pool(name="big", bufs=1))
        wsp = ctx.enter_context(tc.tile_pool(name="wsp", bufs=2))
        ep = ctx.enter_context(tc.tile_pool(name="ep", bufs=2))
        sml = ctx.enter_context(tc.tile_pool(name="sml", bufs=2))
        ps = ctx.enter_context(tc.tile_pool(name="ps", bufs=1, space="PSUM"))

        # ---- constants ----
        ind = cst.tile([128, 4], f32r)
        nc.sync.dma_start(out=ind, in_=dr["ind"])
        indt = cst.tile([4, 128], f32r)
        nc.sync.dma_start(out=indt, in_=dr["indt"])
        ident = cst.tile([128, 128], f32r)
        nc.sync.dma_start(out=ident, in_=dr["ident"])
        onesr = cst.tile([1, 128], f32r)
        nc.sync.dma_start(out=onesr, in_=dr["onesr"])
        biases = {}
        for b in ("bq", "bk", "bv", "bp", "gnw", "gnb"):
            t = cst.tile([128, NL, CC], f32, name=f"c_{b}")
            nc.sync.dma_start(out=t, in_=dr[b])
            biases[b] = t
        epsc = cst.tile([128, 1], f32)
        nc.sync.dma_start(out=epsc, in_=dr["epsc"])
        initb = cst.tile([128, CC], f32)
        nc.sync.dma_start(out=initb, in_=dr["initb"])

        # persistent activations
        x = big.tile([128, CC, L], f32)
        vT = big.tile([128, CC, H, DH + 1], f32r)
        for hh in range(H):
            for sc in range(CC):
                nc.sync.dma_start(out=vT[:, sc, hh, DH:DH + 1], in_=dr["onesc"])

        # ---- init conv1x1 ----
        if True:
            spt = ep.tile([SPEC, L], f32r, tag="E", name="spt")
            nc.sync.dma_start(out=spt, in_=dr["speech"])
            iwt = ep.tile([SPEC, CC, 128], f32r, tag="E", name="iwt")
            nc.sync.dma_start(out=iwt, in_=dr["initw"])
            for mc in range(CC):
                for tp in range(2):
                    pm = ps.tile([128, 512], f32, tag="mm", bufs=4,
                                 name=f"pi{mc}_{tp}")
                    nc.tensor.matmul(pm, lhsT=iwt[:, mc, :],
                                     rhs=spt[:, tp * 512:(tp + 1) * 512],
                                     start=True, stop=True)
                    nc.vector.tensor_scalar(
                        x[:, mc, tp * 512:(tp + 1) * 512], pm,
                        initb[:, mc:mc + 1], None, Alu.add)

        if TAPS:
            nc.sync.dma_start(out=taps["t_xi"], in_=x)
        # ---- layers ----
        for l in range(NL):
            last = (l == NL - 1)
            TW = 16 if last else L     # t-width for q/attn/proj

            if TAPS and l == 1:
                nc.sync.dma_start(out=taps["t_x0"], in_=x)
            # GroupNorm: x -> h (f32r)
            h = big.tile([128, CC, L], f32r, tag="h", name=f"h{l}")
            for cc in range(CC):
                st = sml.tile([128, 2, 6], f32, tag="bst", name=f"st{l}_{cc}")
                for u in range(2):
                    nc.vector.bn_stats(st[:, u, :],
                                       x[:, cc, u * 512:(u + 1) * 512])
                mv = sml.tile([128, 2], f32, tag="mv", name=f"mv{l}_{cc}")
                nc.vector.bn_aggr(mv, st)
                # grhs = [mean, var + mean^2]  (f32r)
                grhs = sml.tile([128, 2], f32r, tag="grhs", name=f"gr{l}_{cc}")
                nc.vector.tensor_copy(out=grhs[:, 0:1], in_=mv[:, 0:1])
                sq = sml.tile([128, 1], f32, tag="sq", name=f"sq{l}_{cc}")
                nc.vector.tensor_tensor(sq, mv[:, 0:1], mv[:, 0:1], Alu.mult)
                nc.vector.tensor_tensor(grhs[:, 1:2], mv[:, 1:2], sq, Alu.add)
                pg = ps.tile([4, 2], f32, tag="av", bufs=2, name=f"pg{l}_{cc}")
                nc.tensor.matmul(pg, lhsT=ind, rhs=grhs, start=True, stop=True)
                # group stats -> [mean_g, rstd_g] (f32r)
                gm = sml.tile([4, 2], f32r, tag="gm", name=f"gm{l}_{cc}")
                nc.vector.tensor_scalar(gm[:, 0:1], pg[:, 0:1],
                                        1.0 / 32, None, Alu.mult)
                ex2 = sml.tile([4, 1], f32, tag="ex2", name=f"ex{l}_{cc}")
                nc.vector.tensor_scalar(ex2, pg[:, 1:2], 1.0 / 32, None,
                                        Alu.mult)
                m2 = sml.tile([4, 1], f32, tag="m2", name=f"m2{l}_{cc}")
                nc.vector.tensor_tensor(m2, gm[:, 0:1], gm[:, 0:1], Alu.mult)
                var = sml.tile([4, 1], f32, tag="var", name=f"va{l}_{cc}")
                nc.vector.tensor_tensor(var, ex2, m2, Alu.subtract)
                sd = sml.tile([4, 1], f32, tag="sd", name=f"sd{l}_{cc}")
                nc.scalar.activation(sd, var, AF.Sqrt, bias=epsc[0:4, :])
                with nc.allow_low_precision(reason="f32r rstd"):
                    nc.vector.reciprocal(gm[:, 1:2], sd)
                pb = ps.tile([128, 2], f32, tag="av", bufs=2, name=f"pb{l}_{cc}")
                nc.tensor.matmul(pb, lhsT=indt, rhs=gm, start=True, stop=True)
                # scale = rstd*gnw ; shift = gnb - mean*scale
                sc_ = sml.tile([128, 1], f32, tag="scl", name=f"sc{l}_{cc}")
                nc.vector.tensor_tensor(sc_, pb[:, 1:2],
                                        biases["gnw"][:, l, cc:cc + 1], Alu.mult)
                ms = sml.tile([128, 1], f32, tag="ms", name=f"ms{l}_{cc}")
                nc.vector.tensor_tensor(ms, pb[:, 0:1], sc_, Alu.mult)
                sh = sml.tile([128, 1], f32, tag="sh", name=f"shf{l}_{cc}")
                nc.vector.tensor_tensor(sh, biases["gnb"][:, l, cc:cc + 1],
                                        ms, Alu.subtract)
                nc.vector.tensor_scalar(h[:, cc, :], x[:, cc, :], sc_, sh,
                                        Alu.mult, Alu.add)

            if TAPS and l == 0:
                nc.sync.dma_start(out=taps["t_h0"], in_=h.bitcast(f32))
            attn = big.tile([128, CC, TW], f32r, tag="attn", name=f"at{l}")

            for half in range(2):
                # q/k/v projections for this half (512 output channels)
                qw = TW
                q = big.tile([128, CC // 2, qw], f32r, tag="q", name=f"q{l}_{half}")
                k = big.tile([128, CC // 2, L], f32r, tag="k", name=f"k{l}_{half}")
                v = big.tile([128, CC // 2, L], f32r, tag="v", name=f"v{l}_{half}")
                for dst, w, bias, dw in ((q, "wq", "bq", qw),
                                         (k, "wk", "bk", L),
                                         (v, "wv", "bv", L)):
                    for mc in range(CC // 2):
                        gmc = half * (CC // 2) + mc
                        ws = wsp.tile([128, CC, 128], f32r, tag="ws",
                                      name=f"w{l}_{half}_{w}_{mc}")
                        nc.sync.dma_start(out=ws, in_=dr[w][l, gmc])
                        for t0 in range(0, dw, 512):
                            tw = min(512, dw - t0)
                            pm = ps.tile([128, tw], f32, tag="mm", bufs=4,
                                         name=f"p{l}_{half}_{w}_{mc}_{t0}")
                            for kc in range(CC):
                                nc.tensor.matmul(pm, lhsT=ws[:, kc, :],
                                                 rhs=h[:, kc, t0:t0 + tw],
                                                 start=(kc == 0),
                                                 stop=(kc == CC - 1))
                            nc.vector.tensor_scalar(
                                dst[:, mc, t0:t0 + tw], pm,
                                biases[bias][:, l, gmc:gmc + 1], None, Alu.add)

                if TAPS and l == 0 and half == 0:
                    nc.sync.dma_start(out=taps["t_q0"], in_=q.bitcast(f32))
                    nc.sync.dma_start(out=taps["t_k0"], in_=k.bitcast(f32))
                # vT per head of this half
                for h8 in range(8):
                    hh = half * 8 + h8
                    lo = (h8 % 2) * 64
                    for sc in range(CC):
                        pt = ps.tile([128, 64], f32r, tag="mm", bufs=4,
                                     name=f"pt{l}_{hh}_{sc}")
                        nc.tensor.transpose(
                            pt, v[lo:lo + 64, h8 // 2, sc * 128:(sc + 1) * 128],
                            ident[lo:lo + 64, lo:lo + 64])
                        nc.vector.tensor_copy(out=vT[:, sc, hh, 0:DH], in_=pt)

                # attention per head
                TB = 16 if last else 256
                for h8 in range(8):
                    hh = half * 8 + h8
                    lo = (h8 % 2) * 64
                    hc = h8 // 2
                    for t0 in range(0, TW, TB):
                        et = ep.tile([128, CC, TB], f32r, tag="E",
                                     name=f"e{l}_{hh}_{t0}")
                        for sc in range(CC):
                            pss = ps.tile([128, TB], f32, tag="s", bufs=2,
                                          name=f"ps{l}_{hh}_{t0}_{sc}")
                            nc.tensor.matmul(
                                pss,
                                lhsT=k[lo:lo + 64, hc, sc * 128:(sc + 1) * 128],
                                rhs=q[lo:lo + 64, hc, t0:t0 + TB],
                                start=True, stop=True)
                            nc.scalar.activation(et[:, sc, :], pss, AF.Exp,
                                                 bias=0.0, scale=0.125)
                        pav = ps.tile([65, TB], f32, tag="av", bufs=2,
                                      name=f"pa{l}_{hh}_{t0}")
                        for sc in range(CC):
                            nc.tensor.matmul(pav, lhsT=vT[:, sc, hh, :],
                                             rhs=et[:, sc, :],
                                             start=(sc == 0), stop=(sc == CC - 1))
                        rec = sml.tile([1, TB], f32r, tag="rec",
                                       name=f"rc{l}_{hh}_{t0}")
                        with nc.allow_low_precision(reason="softmax denom"):
                            nc.vector.reciprocal(rec, pav[64:65, :])
                        pbc = ps.tile([64, TB], f32, tag="mm", bufs=4,
                                      name=f"pb{l}_{hh}_{t0}")
                        nc.tensor.matmul(pbc, lhsT=onesr[:, 0:64], rhs=rec,
                                         start=True, stop=True)
                        rb = sml.tile([64, TB], f32, tag="rb", bufs=1,
                                      name=f"rb{l}_{hh}_{t0}")
                        nc.vector.tensor_copy(out=rb, in_=pbc)
                        # attn channel = hh*64 + c  ->  chunk hh//2, part (hh%2)*64
                        alo = (hh % 2) * 64
                        nc.vector.tensor_tensor(
                            attn[alo:alo + 64, hh // 2, t0:t0 + TB],
                            pav[0:64, :], rb, Alu.mult)

            if TAPS and l == 0:
                nc.sync.dma_start(out=taps["t_at0"], in_=attn.bitcast(f32))
            # proj + residual
            for mc in range(CC):
                ws = wsp.tile([128, CC, 128], f32r, tag="ws",
                              name=f"wp{l}_{mc}")
                nc.sync.dma_start(out=ws, in_=dr["wp"][l, mc])
                for t0 in range(0, TW, 512):
                    tw = min(512, TW - t0)
                    pm = ps.tile([128, tw], f32, tag="mm", bufs=4,
                                 name=f"pp{l}_{mc}_{t0}")
                    for kc in range(CC):
                        nc.tensor.matmul(pm, lhsT=ws[:, kc, :],
                                         rhs=attn[:, kc, t0:t0 + tw],
                                         start=(kc == 0), stop=(kc == CC - 1))
                    nc.vector.scalar_tensor_tensor(
                        out=x[:, mc, t0:t0 + tw], in0=pm,
                        scalar=biases["bp"][:, l, mc:mc + 1],
                        in1=x[:, mc, t0:t0 + tw], op0=Alu.add, op1=Alu.add)

        # ---- output: x[:, :, 0] ----
        o = cst.tile([128, CC], f32)
        nc.vector.tensor_copy(out=o, in_=x[:, :, 0:1].squeeze(-1))
        nc.sync.dma_start(out=out_d.rearrange("(c p) -> p c", p=128), in_=o)

    nc.compile()
    return nc


def _prep(inputs):
    """Host-side weight restaging -> per-core input maps."""
    g = {k: np.asarray(v, np.float32) for k, v in inputs.items()}

    idx = np.arange(3 * C).reshape(H, 3, DH)
    qidx, kidx, vidx = idx[:, 0].ravel(), idx[:, 1].ravel(), idx[:, 2].ravel()

    def stage_w(w):            # w [NL, 1024(out), 1024(in)] -> staged lhsT
        wt = w.transpose(0, 2, 1)                    # [l, in, out]
        return np.ascontiguousarray(
            wt.reshape(NL, CC, 128, CC, 128).transpose(0, 3, 2, 1, 4))

    def stage_b(b):            # [NL, 1024] -> [128, NL, CC]
        return np.ascontiguousarray(
            b.reshape(NL, CC, 128).transpose(2, 0, 1))

    qkv_w, qkv_b = g["qkv_w"], g["qkv_b"]
    common = {
        "wq": stage_w(qkv_w[:, qidx, :]),
        "wk": stage_w(qkv_w[:, kidx, :]),
        "wv": stage_w(qkv_w[:, vidx, :]),
        "wp": stage_w(g["proj_w"]),
        "bq": stage_b(qkv_b[:, qidx]),
        "bk": stage_b(qkv_b[:, kidx]),
        "bv": stage_b(qkv_b[:, vidx]),
        "bp": stage_b(g["proj_b"]),
        "gnw": stage_b(g["gn_w"]),
        "gnb": stage_b(g["gn_b"]),
        "initw": np.ascontiguousarray(g["init_w"].T.reshape(SPEC, CC, 128)),
        "initb": np.ascontiguousarray(g["init_b"].reshape(CC, 128).T),
        "ind": np.equal(np.arange(128)[:, None] // 32,
                        np.arange(4)[None, :]).astype(np.float32),
        "indt": np.equal(np.arange(128)[None, :] // 32,
                         np.arange(4)[:, None]).astype(np.float32),
        "ident": np.eye(128, dtype=np.float32),
        "onesr": np.ones((1, 128), np.float32),
        "onesc": np.ones((128, 1), np.float32),
        "epsc": np.full((128, 1), EPS, np.float32),
    }
    in_maps = []
    for b in range(B):
        m = dict(common)
        m["speech"] = np.ascontiguousarray(g["speech"][b])
        in_maps.append(m)
    return in_maps


def kernel(**inputs):
    global LAST_RESULT
    if "nc" not in _CACHE:
        _CACHE["nc"] = _build()
    nc = _CACHE["nc"]
    in_maps = _prep(inputs)
    res = run_bass_kernel_spmd(nc, in_maps, list(range(NCORES)))
    LAST_RESULT = res
    out = np.stack([res.results[b]["out"] for b in range(B)])
    return out.astype(np.float32)



# revision 4
# speedup vs baseline: 1.3115x; 1.3115x over previous
"""Trainium2 Bass kernel for nn_ConditioningEncoder (6-layer attention encoder).

Strategy: data-parallel over batch (B=8 -> 1 batch element per NeuronCore).
All big matmuls run in bf16 (1 cycle/row on PE, same as f32r, but smaller
SBUF footprint + cheaper transposes); f32 accumulation in PSUM throughout.

Per-core computation (C=1024 channels, L=1024 positions, 16 heads, dh=64):
  x = init_w @ speech + init_b                        [C, L] f32
  6x attention blocks:
    h = GroupNorm32(x) * gn_w + gn_b                  (bf16)
    q, k, v = per-channel projections of h (head-major channel order)
    vT = PE-transpose of v per head pair + ones column (denominator trick)
    chains (head, 512-wide t-block), software-pipelined S/AV on PE:
      S^T[s,t] = k_h^T q_h (PSUM); E = exp(S^T/8) -> bf16 (ACT)
      pav[65,t] = [v;1]^T E accumulated over s-chunks (PE)
      attn_raw <- pav[0:64] (DVE), den row <- pav[64] (DVE)
    pass 2: rec = 1/den (one [16,L] DVE op); broadcast rec to channels via
      sel matmul (PE, 2 heads at once); attn *= recb in-place on Pool engine
    x += proj_w @ attn + proj_b
  out = x[:, 0]

Layer 5 computes q/attention/proj only for t=0..16 (only column 0 returned).
"""
import sys

sys.path.insert(0, "/opt/trn_rl_repo")

from contextlib import ExitStack

import numpy as np
import ml_dtypes

import concourse.bass as bass
import concourse.tile as tile
from concourse import bacc, mybir
from concourse.bass_utils import run_bass_kernel_spmd

f32 = mybir.dt.float32
f32r = mybir.dt.float32r
bf16 = mybir.dt.bfloat16
AF = mybir.ActivationFunctionType
Alu = mybir.AluOpType

B, SPEC, L = 8, 80, 1024
C, H, DH, NL, NG = 1024, 16, 64, 6, 32
CC = C // 128          # channel chunks per full width
EPS = 1e-5
NCORES = 8

LAST_RESULT = None     # test harness reads exec_time from here
_CACHE = {}


def _build():
    nc = bacc.Bacc("TRN2", target_bir_lowering=False, debug=False,
                   num_devices=NCORES)

    dr = {}
    def din(name, shape, dt):
        dr[name] = nc.dram_tensor(name, shape, dt, kind="ExternalInput").ap()

    din("speech", [SPEC, L], f32r)
    din("initw", [SPEC, CC, 128], f32r)
    din("initb", [128, CC], f32)
    for w in ("wq", "wk", "wv", "wp"):
        din(w, [NL, CC, 128, CC, 128], bf16)
    for b in ("bq", "bk", "bv", "bp", "gnw", "gnb"):
        din(b, [128, NL, CC], f32)
    din("ind", [128, 4], f32r)
    din("indt", [4, 128], f32r)
    din("ident", [128, 128], bf16)
    din("onesc", [128, 1], bf16)
    din("sel", [16, 8, 128], bf16)
    din("epsc", [128, 1], f32)
    out_d = nc.dram_tensor("out", [C], f32, kind="ExternalOutput").ap()

    with tile.TileContext(nc) as tc, ExitStack() as ctx:
        cst = ctx.enter_context(tc.tile_pool(name="cst", bufs=1))
        big = ctx.enter_context(tc.tile_pool(name="big", bufs=1))
        wsp = ctx.enter_context(tc.tile_pool(name="wsp", bufs=2))
        ep = ctx.enter_context(tc.tile_pool(name="ep", bufs=2))
        sml = ctx.enter_context(tc.tile_pool(name="sml", bufs=2))
        ps = ctx.enter_context(tc.tile_pool(name="ps", bufs=1, space="PSUM"))

        # ---- constants ----
        ind = cst.tile([128, 4], f32r)
        nc.sync.dma_start(out=ind, in_=dr["ind"])
        indt = cst.tile([4, 128], f32r)
        nc.sync.dma_start(out=indt, in_=dr["indt"])
        ident = cst.tile([128, 128], bf16)
        nc.sync.dma_start(out=ident, in_=dr["ident"])
        sel = cst.tile([16, 8, 128], bf16)
        nc.sync.dma_start(out=sel, in_=dr["sel"])
        biases = {}
        for b in ("bq", "bk", "bv", "bp", "gnw", "gnb"):
            t = cst.tile([128, NL, CC], f32, name=f"c_{b}")
            nc.sync.dma_start(out=t, in_=dr[b])
            biases[b] = t
        epsc = cst.tile([128, 1], f32)
        nc.sync.dma_start(out=epsc, in_=dr["epsc"])
        initb = cst.tile([128, CC], f32)
        nc.sync.dma_start(out=initb, in_=dr["initb"])

        # persistent activations
        x = big.tile([128, CC, L], f32)
        # vT[s, sc, pair, half, 0:64] = v chans; [..., 64] = 1.0 (denom col)
        vT = big.tile([128, CC, 8, 2, 65], bf16)
        for sc in range(CC):
            for p8 in range(8):
                for hp in range(2):
                    nc.sync.dma_start(out=vT[:, sc, p8, hp, 64:65],
                                      in_=dr["onesc"])

        # ---- init conv1x1 ----
        spt = ep.tile([SPEC, L], f32r, tag="E", name="spt")
        nc.sync.dma_start(out=spt, in_=dr["speech"])
        iwt = ep.tile([SPEC, CC, 128], f32r, tag="E", name="iwt")
        nc.sync.dma_start(out=iwt, in_=dr["initw"])
        for mc in range(CC):
            for tp in range(2):
                pm = ps.tile([128, 512], f32, tag="mm", bufs=2,
                             name=f"pi{mc}_{tp}")
                nc.tensor.matmul(pm, lhsT=iwt[:, mc, :],
                                 rhs=spt[:, tp * 512:(tp + 1) * 512],
                                 start=True, stop=True)
                nc.vector.tensor_scalar(
                    x[:, mc, tp * 512:(tp + 1) * 512], pm,
                    initb[:, mc:mc + 1], None, Alu.add)

        # ---- layers ----
        for l in range(NL):
            last = (l == NL - 1)
            TW = 16 if last else L     # t-width for q/attn/proj
            TB = 16 if last else 512   # attention t-block

            # GroupNorm: x -> h (bf16)
            h = big.tile([128, CC, L], bf16, tag="h", name=f"h{l}")
            for cc in range(CC):
                st = sml.tile([128, 2, 6], f32, tag="bst", name=f"st{l}_{cc}")
                for u in range(2):
                    nc.vector.bn_stats(st[:, u, :],
                                       x[:, cc, u * 512:(u + 1) * 512])
                mv = sml.tile([128, 2], f32, tag="mv", name=f"mv{l}_{cc}")
                nc.vector.bn_aggr(mv, st)
                # grhs = [mean, var + mean^2]  (f32r)
                grhs = sml.tile([128, 2], f32r, tag="grhs", name=f"gr{l}_{cc}")
                nc.vector.tensor_copy(out=grhs[:, 0:1], in_=mv[:, 0:1])
                sq = sml.tile([128, 1], f32, tag="sq", name=f"sq{l}_{cc}")
                nc.vector.tensor_tensor(sq, mv[:, 0:1], mv[:, 0:1], Alu.mult)
                nc.vector.tensor_tensor(grhs[:, 1:2], mv[:, 1:2], sq, Alu.add)
                pg = ps.tile([4, 2], f32, tag="pt", bufs=2, name=f"pg{l}_{cc}")
                nc.tensor.matmul(pg, lhsT=ind, rhs=grhs, start=True, stop=True)
                # group stats -> [mean_g, rstd_g] (f32r)
                gm = sml.tile([4, 2], f32r, tag="gm", name=f"gm{l}_{cc}")
                nc.vector.tensor_scalar(gm[:, 0:1], pg[:, 0:1],
                                        1.0 / 32, None, Alu.mult)
                ex2 = sml.tile([4, 1], f32, tag="ex2", name=f"ex{l}_{cc}")
                nc.vector.tensor_scalar(ex2, pg[:, 1:2], 1.0 / 32, None,
                                        Alu.mult)
                m2 = sml.tile([4, 1], f32, tag="m2", name=f"m2{l}_{cc}")
                nc.vector.tensor_tensor(m2, gm[:, 0:1], gm[:, 0:1], Alu.mult)
                var = sml.tile([4, 1], f32, tag="var", name=f"va{l}_{cc}")
                nc.vector.tensor_tensor(var, ex2, m2, Alu.subtract)
                sd = sml.tile([4, 1], f32, tag="sd", name=f"sd{l}_{cc}")
                nc.scalar.activation(sd, var, AF.Sqrt, bias=epsc[0:4, :])
                with nc.allow_low_precision(reason="f32r rstd"):
                    nc.vector.reciprocal(gm[:, 1:2], sd)
                pb = ps.tile([128, 2], f32, tag="pt", bufs=2,
                             name=f"pb{l}_{cc}")
                nc.tensor.matmul(pb, lhsT=indt, rhs=gm, start=True, stop=True)
                # scale = rstd*gnw ; shift = gnb - mean*scale
                sc_ = sml.tile([128, 1], f32, tag="scl", name=f"sc{l}_{cc}")
                nc.vector.tensor_tensor(sc_, pb[:, 1:2],
                                        biases["gnw"][:, l, cc:cc + 1], Alu.mult)
                ms = sml.tile([128, 1], f32, tag="ms", name=f"ms{l}_{cc}")
                nc.vector.tensor_tensor(ms, pb[:, 0:1], sc_, Alu.mult)
                sh = sml.tile([128, 1], f32, tag="sh", name=f"shf{l}_{cc}")
                nc.vector.tensor_tensor(sh, biases["gnb"][:, l, cc:cc + 1],
                                        ms, Alu.subtract)
                nc.vector.tensor_scalar(h[:, cc, :], x[:, cc, :], sc_, sh,
                                        Alu.mult, Alu.add)

            # q/k/v projections (full 16 heads at once)
            q = big.tile([128, CC, TW], bf16, tag="q", name=f"q{l}")
            k = big.tile([128, CC, L], bf16, tag="k", name=f"k{l}")
            v = big.tile([128, CC, L], bf16, tag="v", name=f"v{l}")
            for dst, w, bias, dw in ((q, "wq", "bq", TW),
                                     (k, "wk", "bk", L),
                                     (v, "wv", "bv", L)):
                for mc in range(CC):
                    ws = wsp.tile([128, CC, 128], bf16, tag="ws",
                                  name=f"w{l}_{w}_{mc}")
                    nc.sync.dma_start(out=ws, in_=dr[w][l, mc])
                    for t0 in range(0, dw, 512):
                        tw = min(512, dw - t0)
                        pm = ps.tile([128, tw], f32, tag="mm", bufs=2,
                                     name=f"p{l}_{w}_{mc}_{t0}")
                        for kc in range(CC):
                            nc.tensor.matmul(pm, lhsT=ws[:, kc, :],
                                             rhs=h[:, kc, t0:t0 + tw],
                                             start=(kc == 0),
                                             stop=(kc == CC - 1))
                        nc.vector.tensor_scalar(
                            dst[:, mc, t0:t0 + tw], pm,
                            biases[bias][:, l, mc:mc + 1], None, Alu.add)

            # vT: transpose v per head pair (both heads of chunk p at once)
            for p8 in range(8):
                for sc in range(CC):
                    pt = ps.tile([128, 2, 64], bf16, tag="pt", bufs=2,
                                 name=f"pt{l}_{p8}_{sc}")
                    nc.tensor.transpose(
                        pt, v[:, p8, sc * 128:(sc + 1) * 128], ident)
                    nc.scalar.activation(vT[:, sc, p8, :, 0:64], pt, AF.Copy)

            # attention chains: (head, t-block), S/AV software-pipelined
            attn = big.tile([128, CC, TW], bf16, tag="attn", name=f"at{l}")
            den = sml.tile([16, TW], f32, tag="den", bufs=1, name=f"dn{l}")

            def do_S(hh, t0, et):
                lo = (hh % 2) * 64
                hc = hh // 2
                for sc in range(CC):
                    pss = ps.tile([128, TB], f32, tag="s", bufs=2,
                                  name=f"ps{l}_{hh}_{t0}_{sc}")
                    nc.tensor.matmul(
                        pss,
                        lhsT=k[lo:lo + 64, hc, sc * 128:(sc + 1) * 128],
                        rhs=q[lo:lo + 64, hc, t0:t0 + TB],
                        start=True, stop=True)
                    nc.scalar.activation(et[:, sc, :], pss, AF.Exp,
                                         bias=0.0, scale=0.125)

            def do_AV(hh, t0, et):
                pav = ps.tile([65, TB], f32, tag="av", bufs=2,
                              name=f"pa{l}_{hh}_{t0}")
                for sc in range(CC):
                    nc.tensor.matmul(pav,
                                     lhsT=vT[:, sc, hh // 2, hh % 2, :],
                                     rhs=et[:, sc, :],
                                     start=(sc == 0), stop=(sc == CC - 1))
                alo = (hh % 2) * 64
                nc.vector.tensor_copy(
                    out=attn[alo:alo + 64, hh // 2, t0:t0 + TB],
                    in_=pav[0:64, :])
                stg = sml.tile([128, TB], f32, tag="stg", bufs=2,
                               name=f"sg{l}_{hh}_{t0}")
                nc.vector.tensor_copy(out=stg[64:65, :], in_=pav[64:65, :])
                nc.sync.dma_start(out=den[hh:hh + 1, t0:t0 + TB],
                                  in_=stg[64:65, :])

            prev = None
            for hh in range(H):
                for t0 in range(0, TW, TB):
                    et = ep.tile([128, CC, TB], bf16, tag="E",
                                 name=f"e{l}_{hh}_{t0}")
                    do_S(hh, t0, et)
                    if prev is not None:
                        do_AV(*prev)
                    prev = (hh, t0, et)
            do_AV(*prev)

            # pass 2: normalize attn by softmax denominator
            rec = sml.tile([16, TW], bf16, tag="rec", bufs=1, name=f"rc{l}")
            with nc.allow_low_precision(reason="softmax denom"):
                nc.vector.reciprocal(rec, den)
            for p8 in range(8):
                for t0 in range(0, TW, TB):
                    pbc = ps.tile([128, TB], f32, tag="s", bufs=2,
                                  name=f"pbc{l}_{p8}_{t0}")
                    nc.tensor.matmul(pbc, lhsT=sel[:, p8, :],
                                     rhs=rec[:, t0:t0 + TB],
                                     start=True, stop=True)
                    recb = sml.tile([128, TB], bf16, tag="recb", bufs=2,
                                    name=f"rb{l}_{p8}_{t0}")
                    nc.scalar.activation(recb, pbc, AF.Copy)
                    for hp in range(2):
                        rows = slice(hp * 64, hp * 64 + 64)
                        nc.gpsimd.tensor_tensor(
                            attn[rows, p8, t0:t0 + TB],
                            attn[rows, p8, t0:t0 + TB],
                            recb[rows, :], Alu.mult)

            # proj + residual
            for mc in range(CC):
                ws = wsp.tile([128, CC, 128], bf16, tag="ws",
                              name=f"wp{l}_{mc}")
                nc.sync.dma_start(out=ws, in_=dr["wp"][l, mc])
                for t0 in range(0, TW, 512):
                    tw = min(512, TW - t0)
                    pm = ps.tile([128, tw], f32, tag="mm", bufs=2,
                                 name=f"pp{l}_{mc}_{t0}")
                    for kc in range(CC):
                        nc.tensor.matmul(pm, lhsT=ws[:, kc, :],
                                         rhs=attn[:, kc, t0:t0 + tw],
                                         start=(kc == 0), stop=(kc == CC - 1))
                    nc.vector.scalar_tensor_tensor(
                        out=x[:, mc, t0:t0 + tw], in0=pm,
                        scalar=biases["bp"][:, l, mc:mc + 1],
                        in1=x[:, mc, t0:t0 + tw], op0=Alu.add, op1=Alu.add)

        # ---- output: x[:, :, 0] ----
        o = cst.tile([128, CC], f32)
        nc.vector.tensor_copy(out=o, in_=x[:, :, 0:1].squeeze(-1))
        nc.sync.dma_start(out=out_d.rearrange("(c p) -> p c", p=128), in_=o)

    nc.compile()
    return nc


def _prep(inputs):
    """Host-side weight restaging -> per-core input maps."""
    g = {k: np.asarray(v, np.float32) for k, v in inputs.items()}
    bf = ml_dtypes.bfloat16

    idx = np.arange(3 * C).reshape(H, 3, DH)
    qidx, kidx, vidx = idx[:, 0].ravel(), idx[:, 1].ravel(), idx[:, 2].ravel()

    def stage_w(w):            # w [NL, 1024(out), 1024(in)] -> staged lhsT
        wt = w.transpose(0, 2, 1)                    # [l, in, out]
        return np.ascontiguousarray(
            wt.reshape(NL, CC, 128, CC, 128).transpose(0, 3, 2, 1, 4)
        ).astype(bf)

    def stage_b(b):            # [NL, 1024] -> [128, NL, CC]
        return np.ascontiguousarray(
            b.reshape(NL, CC, 128).transpose(2, 0, 1))

    sel = np.zeros((16, 8, 128), np.float32)
    for hh in range(16):
        for p8 in range(8):
            for c in range(128):
                if hh == 2 * p8 + c // 64:
                    sel[hh, p8, c] = 1.0

    qkv_w, qkv_b = g["qkv_w"], g["qkv_b"]
    common = {
        "wq": stage_w(qkv_w[:, qidx, :]),
        "wk": stage_w(qkv_w[:, kidx, :]),
        "wv": stage_w(qkv_w[:, vidx, :]),
        "wp": stage_w(g["proj_w"]),
        "bq": stage_b(qkv_b[:, qidx]),
        "bk": stage_b(qkv_b[:, kidx]),
        "bv": stage_b(qkv_b[:, vidx]),
        "bp": stage_b(g["proj_b"]),
        "gnw": stage_b(g["gn_w"]),
        "gnb": stage_b(g["gn_b"]),
        "initw": np.ascontiguousarray(g["init_w"].T.reshape(SPEC, CC, 128)),
        "initb": np.ascontiguousarray(g["init_b"].reshape(CC, 128).T),
        "ind": np.equal(np.arange(128)[:, None] // 32,
                        np.arange(4)[None, :]).astype(np.float32),
        "indt": np.equal(np.arange(128)[None, :] // 32,
                         np.arange(4)[:, None]).astype(np.float32),
        "ident": np.eye(128, dtype=np.float32).astype(bf),
        "onesc": np.ones((128, 1), np.float32).astype(bf),
        "sel": sel.astype(bf),
        "epsc": np.full((128, 1), EPS, np.float32),
    }
    in_maps = []
    for b in range(B):
        m = dict(common)
        m["speech"] = np.ascontiguousarray(g["speech"][b])
        in_maps.append(m)
    return in_maps


def kernel(**inputs):
    global LAST_RESULT
    if "nc" not in _CACHE:
        _CACHE["nc"] = _build()
    nc = _CACHE["nc"]
    in_maps = _prep(inputs)
    res = run_bass_kernel_spmd(nc, in_maps, list(range(NCORES)))
    LAST_RESULT = res
    out = np.stack([res.results[b]["out"] for b in range(B)])
    return out.astype(np.float32)
